# revision 38
# baseline (speedup 1.0000x reference)
"""Trainium2 Bass kernel for a 3-layer binary-weight MLP.

Problem (nn_MLP_56779467653689):
    x: [8192, 1024] f32
    h = relu(s0 * (x @ W0)) * 2      W0 = 2*k0-1  in {-1,+1}, [1024, 4096]
    h = relu(s1 * (h @ W1)) * 2      W1 [4096, 4096]
    out = s2 * (h @ W2)              W2 [4096, 1024]

Strategy: pure data-parallel over tokens across 8 NeuronCores (1024
tokens/core). Per core, activations live in SBUF as [features, tokens]
(features on partitions) so layers chain with no transposes.

Precision plan (PE roofline is the bottleneck; measured 221 ns per
128-contraction fp16/bf16 matmul vs 225 ns per 256-contraction fp8e4
DoubleRow matmul => DR is 1.97x): run the first N_DR*256 contraction rows
of layer 2 in fp8e4 DoubleRow (moving pairs = 2 k-planes per partition),
everything else in fp16 (exact for +-1 weights, ~2x less quantization
noise than bf16 on x/h). e4m3 quantization of h1 costs ~2.67e-2 rel err
per fully-quantized layer; with N_DR=8 of 16 tiles the simulated
end-to-end rel err is 1.89e-2 against the 2e-2 gate, and the DR half of
L2 runs at ~2x. DoubleRow with fp8 is exact for +-1 weights (products
pass through, fp32 accumulate), so the only error source is the e4m3
eviction rounding, which matches ml_dtypes RNE bit-for-bit (validated on
HW).
"""

from contextlib import ExitStack

import ml_dtypes
import numpy as np

P = 128
TOKENS = 8192
D_IN = 1024
D_H = 4096
D_OUT = 1024
N_CORES = 8
TOK_PER_CORE = TOKENS // N_CORES  # 1024
TOK_TILE = 512
NT = TOK_PER_CORE // TOK_TILE  # 2

N_DR = 8  # 256-row fp8 DoubleRow tiles in L2 (of 16); rest fp16
EXTRA_DR_T0 = True  # 9th DR tile, applied to the t=0 token half only
N_DR_W = N_DR + (1 if EXTRA_DR_T0 else 0)  # DR weight tiles to pack/stream
N_H1Q = 2 * N_DR  # L1 output strips evicted to fp8 pair-tiles (both halves)
N_H1H = D_H // P - N_H1Q  # L1 output strips evicted to fp16
N_WARM = 14  # dummy warm-up matmuls to lift the PE HAM clock-gate early

F16 = np.float16
F8 = ml_dtypes.float8_e4m3  # IEEE-style, max 240 == TRN float8e4

# Set TRACE=True (from test.py) to profile; LAST_EXEC_TIME_NS then holds the
# max per-core HW exec time of the most recent kernel() call.
TRACE = False
TRACE_CORES = None
LAST_EXEC_TIME_NS = None
LAST_RESULT = None

_cache = {}


def _raw_matmul(nc, out, lhsT, rhs, perf_mode=None, start=True, stop=True):
    """nc.tensor.matmul minus the fp8e4/e5-only perf-mode dtype assert."""
    import concourse.mybir as mybir

    eng = nc.tensor
    keep_dims = {0}
    if perf_mode is not None:
        keep_dims.add(1)
    ifmap_ap = eng.lower_ap(rhs.opt(keep_dims), opt=False)
    weights_ap = eng.lower_ap(lhsT.opt(keep_dims), opt=False,
                              for_matmul_weights=True)
    out_ap = eng.lower_ap(out)
    return eng.add_instruction(
        mybir.InstMatmult(
            name=nc.get_next_instruction_name(),
            replication_resolution=0,
            replication_shift_amnt=0,
            replication_num_rows=0,
            start_tensor_calc=start,
            stop_tensor_calc=stop,
            ins=[ifmap_ap, weights_ap],
            outs=[out_ap],
            perf_mode=perf_mode,
            is_transpose=None,
            ifmap_quant_offset=None,
            weights_quant_offset=None,
            bass_skip_group_check=False,
            tile_position=(0, 0),
            tile_size=(128, 128),
        )
    )


def _prune_dma_waits(nc, max_waits=1):
    """Drop transitively-implied waits from DMA instructions.

    DMA queue-entry descriptors hold a single sync wait; Tile's sem
    assignment is per-proc minimal but not transitively minimal across
    procs, so a recycled SBUF slot's DMA can carry WAR (engine) + WAW
    (prev slot writer's DMA lane) + lane-recycle waits = 3. The WAW (and
    often the recycle) wait is implied by the engine wait: the readers
    counted by the WAR threshold themselves waited on those DMAs.

    Soundness: a wait (s >= v) on instruction I is dropped only when the
    completion clocks implied by I's *other* waits already guarantee
    cumulative increments of s reached v. Completion clocks are built
    forward over the scheduled BIR order giving same-stream predecessor
    credit only to in-order engines (PE/ACT/DVE/SP), never to DMA lanes
    or Pool. Unrecognized wait/update modes contribute no credit, so
    unknowns can only inhibit pruning, never enable it.
    """
    import bisect

    import bass_rust

    IN_ORDER_ENGINES = {
        "EngineType.PE",
        "EngineType.Activation",
        "EngineType.DVE",
        "EngineType.SP",
    }

    sem_hist = {}  # sem -> ([cumulative values], [clocks at completion])
    sem_cum = {}  # sem -> cumulative increments so far
    eng_clock = {}  # engine -> completion clock of last instruction
    poisoned = set()  # sems with non-monotonic updates: no credit

    def cc(sem, val):
        """Completion clock implied by observing sem >= val, or None."""
        if sem in poisoned:
            return None
        hist = sem_hist.get(sem)
        if not hist or hist[0][-1] < val:
            return None
        return hist[1][bisect.bisect_left(hist[0], val)]

    def merge(dst, src):
        for k, v in src.items():
            if dst.get(k, 0) < v:
                dst[k] = v

    pruned = 0
    for bb in nc.m.functions[0].blocks:
        for inst in bb.instructions:
            si = inst.sync_info
            waits = list(si.on_wait or []) if si is not None else []
            ups = list(si.on_update or []) if si is not None else []
            is_dma = type(inst).__name__ == "InstDMACopy"

            clock = {}
            if not is_dma:
                prev = eng_clock.get(str(inst.engine))
                if prev is not None and str(inst.engine) in IN_ORDER_ENGINES:
                    merge(clock, prev)
            for w in waits:
                if w.wait_mode == "sem-ge-imm" and w.wait_value is not None:
                    c = cc(w.ant_name, w.wait_value)
                    if c is not None:
                        merge(clock, c)

            # Per-encoding wait budgets: DMA queue entries hold 1 wait;
            # engine instructions hold 2. Drain/EventSemaphore/control flow
            # are lowered specially by walrus — leave them alone.
            tname = type(inst).__name__
            if is_dma:
                cap = max_waits
            elif tname in ("InstDrain", "InstEventSemaphore", "InstCall",
                           "InstUnconditionalBranch", "InstISA"):
                cap = None
            else:
                cap = 2

            if cap is not None and len(waits) > cap:
                kept = list(waits)
                changed = True
                while len(kept) > cap and changed:
                    changed = False
                    for w in list(kept):
                        if w.wait_mode != "sem-ge-imm" or w.wait_value is None:
                            continue
                        implied = {}
                        provable = True
                        for o in kept:
                            if o is w:
                                continue
                            if o.wait_mode != "sem-ge-imm" or o.wait_value is None:
                                provable = False
                                break
                            c = cc(o.ant_name, o.wait_value)
                            if c is None:
                                provable = False
                                break
                            merge(implied, c)
                        if provable and implied.get(w.ant_name, 0) >= w.wait_value:
                            kept.remove(w)
                            pruned += 1
                            changed = True
                            break
                # Anything still over budget is left for Bacc's
                # generate_event_semaphores pass to split legally.
                if len(kept) != len(waits):
                    inst.sync_info = bass_rust.SyncInfo(on_wait=kept, on_update=ups)

            own = {}
            for u in ups:
                if u.update_mode not in ("sem-inc", "sem-add-imm"):
                    poisoned.add(u.ant_name)
                    continue
                inc = 1 if u.update_mode == "sem-inc" else u.update_value
                if inc is None:
                    poisoned.add(u.ant_name)
                    continue
                sem = u.ant_name
                sem_cum[sem] = sem_cum.get(sem, 0) + inc
                own[sem] = sem_cum[sem]
            merge(clock, own)
            for sem, cum in own.items():
                vals, clocks = sem_hist.setdefault(sem, ([], []))
                vals.append(cum)
                clocks.append(clock)
            if not is_dma:
                eng_clock[str(inst.engine)] = clock
    return pruned


def _build(a0, a1, a2):
    """Build the SPMD single-core program (same NEFF on all 8 cores)."""
    import concourse.mybir as mybir
    import concourse.tile as tile
    from concourse import bacc

    # Bacc (not plain Bass): its finalize() runs the wait-legalization
    # passes (move_matmul_waits_to_ldweights, generate_event_semaphores)
    # that split multi-wait instructions to the 1-wait HW encoding.
    nc = bacc.Bacc(
        "TRN2",
        target_bir_lowering=False,
        debug=False,
        enable_asserts=False,
        num_devices=N_CORES,
    )
    f16 = mybir.dt.float16
    f8 = mybir.dt.float8e4
    f32 = mybir.dt.float32
    DRM = mybir.MatmulPerfMode.DoubleRow

    xt = nc.dram_tensor("xt", [D_IN, TOK_PER_CORE], f16, kind="ExternalInput")
    w0p = nc.dram_tensor("w0p", [D_H // P, P, D_IN], f16, kind="ExternalInput")
    w1q = nc.dram_tensor("w1q", [D_H // P, P, N_DR_W * 256], f8,
                         kind="ExternalInput")
    w1h = nc.dram_tensor("w1h", [D_H // P, P, N_H1H * P], f16,
                         kind="ExternalInput")
    w2p = nc.dram_tensor("w2p", [D_OUT // P, P, D_H], f16, kind="ExternalInput")
    outt = nc.dram_tensor("outt", [D_OUT, TOK_PER_CORE], f32, kind="ExternalOutput")

    relu = mybir.ActivationFunctionType.Relu

    def tsl(t):
        return slice(t * TOK_TILE, (t + 1) * TOK_TILE)

    with tile.TileContext(nc) as tc, ExitStack() as ctx:
        xpool = ctx.enter_context(tc.tile_pool(name="xp", bufs=1))
        h1pool = ctx.enter_context(tc.tile_pool(name="h1p", bufs=1))
        h2pool = ctx.enter_context(tc.tile_pool(name="h2p", bufs=1))
        wpool = ctx.enter_context(tc.tile_pool(name="wp", bufs=4))
        w0pool = ctx.enter_context(tc.tile_pool(name="w0pl", bufs=6))
        w2pool = ctx.enter_context(tc.tile_pool(name="w2p", bufs=2))
        opool = ctx.enter_context(tc.tile_pool(name="op", bufs=3))
        pspool = ctx.enter_context(tc.tile_pool(name="psp", bufs=8, space="PSUM"))

        # x as per-j half-tiles in consumption order (t=0 first). The t=0
        # half is split across the ACT queue (j 0-3) and the SP queue
        # (j 4-7, behind only w0 strip 0) so it lands in ~half the time;
        # the t=1 half follows on ACT. (gpsimd's queue is unusable here:
        # ~10us cold init, and its teardown drain lengthens the tail.)
        x_half = [[None] * NT for _ in range(D_IN // P)]
        with tc.high_priority():
            # Pin the startup-critical transfers to the front of their
            # queues: without this the scheduler interleaves them behind
            # later DMAs and the first chain stalls until ~11us.
            w0_first = w0pool.tile([P, D_IN], f16, tag="w0", name="w0_0_0")
            nc.sync.dma_start(out=w0_first[:], in_=w0p[0])
            for t in range(NT):
                for j in range(D_IN // P):
                    h = xpool.tile([P, TOK_TILE], f16, tag=f"x{j}_{t}",
                                   name=f"x_{j}_{t}")
                    q = nc.sync if (t == 0 and j >= 4) else nc.scalar
                    q.dma_start(out=h[:], in_=xt[j * P : (j + 1) * P, tsl(t)])
                    x_half[j][t] = h

        # Warm-up: dummy matmuls on the first x tile while the rest of the
        # startup DMA is in flight. The PE HAM clock-gate only lifts to
        # 8/8 after ~3.4us of sustained busy; without these the first
        # ~20us of real matmuls run at 1.2 GHz (measured ~10us penalty).
        ps_warm = pspool.tile([P, TOK_TILE], f32, tag="ps", name="ps_warm")
        for i in range(N_WARM):
            nc.tensor.matmul(
                ps_warm[:], x_half[0][0][:, 0:P], x_half[0][0][:],
                start=True, stop=True,
            )

        # h1: first N_DR pair-tiles hold fp8 planes (L1 strips 2r, 2r+1),
        # rest fp16. h2: all fp16. With EXTRA_DR_T0, strips 16/17 are
        # additionally evicted to a t=0-only fp8 pair-tile (h1q8).
        h1q_tiles = [
            h1pool.tile([P, 2, TOK_PER_CORE], f8, tag=f"h1q_{r}", name=f"h1q_{r}")
            for r in range(N_DR)
        ]
        h1q8 = (
            h1pool.tile([P, 2, TOK_TILE], f8, tag="h1q8", name="h1q8")
            if EXTRA_DR_T0 else None
        )
        h1h_tiles = [
            h1pool.tile([P, TOK_PER_CORE], f16, tag=f"h1h_{j}", name=f"h1h_{j}")
            for j in range(N_H1H)
        ]
        h2_tiles = [
            h2pool.tile([P, TOK_PER_CORE], f16, tag=f"h2_{n}", name=f"h2_{n}")
            for n in range(D_H // P)
        ]

        # ---- Layer 1 (fp16). The first T0_FIRST strips run their t=0
        # chain only (the t=0 half of x is all they need, so sustained
        # compute starts while the t=1 half is still in flight), then
        # their t=1 chains, then the rest t_outer per strip. Only the
        # first T0_FIRST w0 strips are streamed twice (+2 MB).
        T0_FIRST = 8

        def l1_evict(n, t):
            if n < N_H1Q:
                return h1q_tiles[n // 2][:, n % 2, tsl(t)]
            if EXTRA_DR_T0 and t == 0 and n < N_H1Q + 2:
                return h1q8[:, n - N_H1Q, :]
            return h1h_tiles[n - N_H1Q][:, tsl(t)]

        def l1_chain(w, n, t, jorder=None):
            jorder = jorder or list(range(D_IN // P))
            ps = pspool.tile([P, TOK_TILE], f32, tag="ps", name=f"ps0_{n}_{t}")
            for i, j in enumerate(jorder):
                nc.tensor.matmul(
                    ps[:],
                    w[:, j * P : (j + 1) * P],
                    x_half[j][t][:],
                    start=(i == 0),
                    stop=(i == D_IN // P - 1),
                )
            nc.scalar.activation(l1_evict(n, t), ps[:], relu, scale=a0)

        def l1_w(n, t):
            if n == 0 and t == 0:
                return w0_first
            w = w0pool.tile([P, D_IN], f16, tag="w0", name=f"w0_{n}_{t}")
            nc.sync.dma_start(out=w[:], in_=w0p[n])
            return w

        # The first chains consume x tiles in their DMA-landing order
        # (j 0-3 on ACT, j 4-7 on SP, interleaved by arrival) so the first
        # chain is never j-blocked while later tiles stream in.
        J_ARRIVAL = [0, 4, 1, 5, 2, 6, 3, 7]
        for n in range(T0_FIRST):
            l1_chain(l1_w(n, 0), n, 0, jorder=J_ARRIVAL)
        for n in range(T0_FIRST, D_H // P):
            w = l1_w(n, 0)
            for t in range(NT):
                l1_chain(w, n, t)
        # t=1 chains of the first strips run LAST: their w0 re-streams ride
        # the SP queue during L1's long slack window instead of colliding
        # with the t=0 w0 stream (measured 2.1us PE gap + HAM re-throttle).
        for n in range(T0_FIRST):
            l1_chain(l1_w(n, 1), n, 1)

        # ---- Layer 2: N_DR fp8-DoubleRow 256-row tiles + N_H1H fp16
        # 128-row tiles per accumulation chain. t-inner alternates PSUM
        # banks, which measures ~0.7 ns/MM faster than same-bank runs.
        for n in range(D_H // P):
            wq = wpool.tile([P, N_DR_W * 256], f8, tag="w1q", name=f"w1q_{n}")
            # ACT queue: keeps the L2 fp8 stream off the SP queue, which
            # otherwise delays the L1 w0 strips (measured 5us LDW stall).
            nc.scalar.dma_start(out=wq[:], in_=w1q[n])
            wh = wpool.tile([P, N_H1H * P], f16, tag="w1h", name=f"w1h_{n}")
            # ACT queue too: early w1h strips on SP delayed the L1 w0
            # stream (~2us LDW stalls at the t_outer transition).
            nc.scalar.dma_start(out=wh[:], in_=w1h[n])
            pss = [
                pspool.tile([P, TOK_TILE], f32, tag="ps", name=f"ps1_{n}_{t}")
                for t in range(NT)
            ]
            for r in range(N_DR_W):
                wap = wq[:, r * 256 : (r + 1) * 256].rearrange(
                    "p (two c) -> p two c", two=2
                )
                for t in range(NT):
                    if r == N_DR:  # t=0-only extra tile
                        if t != 0:
                            continue
                        mv = h1q8[:, :, :]
                    else:
                        mv = h1q_tiles[r][:, :, tsl(t)]
                    _raw_matmul(
                        nc, pss[t][:], wap, mv,
                        perf_mode=DRM, start=(r == 0), stop=False,
                    )
            for j in range(N_H1H):
                for t in range(NT):
                    if EXTRA_DR_T0 and t == 0 and j < 2:
                        continue  # rows 2048..2303 covered by the DR tile
                    nc.tensor.matmul(
                        pss[t][:],
                        wh[:, j * P : (j + 1) * P],
                        h1h_tiles[j][:, tsl(t)],
                        start=(N_DR == 0 and j == 0),
                        stop=(j == N_H1H - 1),
                    )
            for t in range(NT):
                nc.scalar.activation(h2_tiles[n][:, tsl(t)], pss[t][:], relu,
                                     scale=a1)

        # ---- Layer 3 (fp16), streaming the output out per token half.
        for n in range(D_OUT // P):
            w = w2pool.tile([P, D_H], f16, tag="w2", name=f"w2_{n}")
            nc.sync.dma_start(out=w[:], in_=w2p[n])
            pss = [
                pspool.tile([P, TOK_TILE], f32, tag="ps", name=f"ps2_{n}_{t}")
                for t in range(NT)
            ]
            def l3_evict(t):
                # Chunked evictions on alternating engines (ACT / DVE) with
                # the output DMA split over two queues: the final strip's
                # eviction+DMA pipeline shortens the kernel tail.
                half = TOK_TILE // 2
                for c in range(2):
                    o = opool.tile([P, half], f32, tag="o", name=f"o_{n}_{t}_{c}")
                    src = pss[t][:, c * half : (c + 1) * half]
                    if (2 * t + c) % 2 == 0:
                        nc.scalar.mul(o[:], src, a2)
                    else:
                        nc.vector.tensor_scalar_mul(o[:], src, a2)
                    dq = nc.scalar if c % 2 == 0 else nc.sync
                    dq.dma_start(
                        out=outt[n * P : (n + 1) * P,
                                 t * TOK_TILE + c * half : t * TOK_TILE + (c + 1) * half],
                        in_=o[:],
                    )

            # Last strip runs t_outer: its t=0 eviction+DMA overlap the
            # t=1 chain, so only one half's eviction trails the final MM.
            last = n == D_OUT // P - 1
            if last:
                order = [(t, j) for t in range(NT) for j in range(D_H // P)]
            else:
                order = [(t, j) for j in range(D_H // P) for t in range(NT)]
            for t, j in order:
                nc.tensor.matmul(
                    pss[t][:],
                    w[:, j * P : (j + 1) * P],
                    h2_tiles[j][:, tsl(t)],
                    start=(j == 0),
                    stop=(j == D_H // P - 1),
                )
                if last and j == D_H // P - 1:
                    l3_evict(t)
            if not last:
                for t in range(NT):
                    l3_evict(t)

    _prune_dma_waits(nc)
    nc.finalize()
    return nc


def _pack_w_f16(k):
    """Bool [K, N] -> f16 +-1 packed [N/P, P, K]: strip n, partition p,
    free j*P+c  <-  W[j*P+p, n*P+c] (partition = contraction for lhsT)."""
    K, N = k.shape
    w = np.where(k, np.float32(1.0), np.float32(-1.0)).astype(F16)
    return np.ascontiguousarray(
        w.reshape(K // P, P, N // P, P).transpose(2, 1, 0, 3).reshape(N // P, P, K)
    )


def _pack_w1(k1):
    """W1 split: rows [0, N_DR_W*256) -> fp8 DoubleRow pair layout
    [N/P, P, r*256 + plane*128 + c] <- W[256r + 128*plane + p, n*128 + c];
    rows [N_DR*256, 4096) -> fp16 strip layout like _pack_w_f16. With
    EXTRA_DR_T0 rows 2048..2303 are packed in BOTH (fp8 serves the t=0
    token half, fp16 the t=1 half)."""
    w = np.where(k1, np.float32(1.0), np.float32(-1.0))
    wa = w[: N_DR_W * 256].astype(F8)
    wb = w[N_DR * 256 :].astype(F16)
    n_t = wa.shape[1] // P
    # [r, plane, p, n, c] -> [n, p, r, plane, c]
    w1q = np.ascontiguousarray(
        wa.reshape(N_DR_W, 2, P, n_t, P).transpose(3, 2, 0, 1, 4).reshape(n_t, P, -1)
    )
    kb = wb.shape[0]
    w1h = np.ascontiguousarray(
        wb.reshape(kb // P, P, n_t, P).transpose(2, 1, 0, 3).reshape(n_t, P, kb)
    )
    return w1q, w1h


def _enable_ntff_trace():
    """Best-effort plumbing for trace=True under axon in this image.

    The image's ``antenv`` lacks the ``axon_hooks`` shim that
    ``trn_agent_boot`` would normally register the NTFF profile hook
    into, and there is no artifact bucket — stub both.
    """
    import sys
    import types

    import concourse.bass_utils as bu

    bu.upload_artifacts = lambda tmpdir: tmpdir
    try:
        from antenv import axon_hooks
    except ImportError:
        import antenv

        axon_hooks = types.ModuleType("antenv.axon_hooks")
        _state = {"hook": None}
        axon_hooks.set_axon_ntff_profile_hook = lambda h: _state.__setitem__(
            "hook", h
        )
        axon_hooks.get_axon_ntff_profile_hook = lambda: _state["hook"]
        sys.modules["antenv.axon_hooks"] = axon_hooks
        antenv.axon_hooks = axon_hooks
    if axon_hooks.get_axon_ntff_profile_hook() is None:
        from trn_agent_boot.trn_boot import _ntff_profile_via_ctypes

        axon_hooks.set_axon_ntff_profile_hook(
            _ntff_profile_via_ctypes("/opt/axon/libaxon_pjrt.so")
        )


def kernel(x, k0, k1, k2, s0, s1, s2):
    global LAST_EXEC_TIME_NS, LAST_RESULT
    from concourse.bass_utils import run_bass_kernel_spmd

    if TRACE:
        _enable_ntff_trace()

    x = np.asarray(x)
    a0 = 2.0 * float(np.asarray(s0))
    a1 = 2.0 * float(np.asarray(s1))
    a2 = float(np.asarray(s2))

    key = (a0, a1, a2)
    if key not in _cache:
        _cache[key] = _build(a0, a1, a2)
    nc = _cache[key]

    w0p = _pack_w_f16(np.asarray(k0))
    w1qp, w1hp = _pack_w1(np.asarray(k1))
    w2p = _pack_w_f16(np.asarray(k2))

    in_maps = []
    for i in range(N_CORES):
        xs = x[i * TOK_PER_CORE : (i + 1) * TOK_PER_CORE].astype(F16)
        in_maps.append(
            {
                "xt": np.ascontiguousarray(xs.T),
                "w0p": w0p,
                "w1q": w1qp,
                "w1h": w1hp,
                "w2p": w2p,
            }
        )

    res = run_bass_kernel_spmd(
        nc, in_maps, list(range(N_CORES)), trace=TRACE, trace_cores=TRACE_CORES
    )
    LAST_EXEC_TIME_NS = res.exec_time_ns
    LAST_RESULT = res
    out = np.concatenate(
        [res.results[i]["outt"].T for i in range(N_CORES)], axis=0
    )
    return np.ascontiguousarray(out)


# revision 39
# speedup vs baseline: 1.0182x; 1.0182x over previous
"""Trainium2 Bass kernel for a 3-layer binary-weight MLP.

Problem (nn_MLP_56779467653689):
    x: [8192, 1024] f32
    h = relu(s0 * (x @ W0)) * 2      W0 = 2*k0-1  in {-1,+1}, [1024, 4096]
    h = relu(s1 * (h @ W1)) * 2      W1 [4096, 4096]
    out = s2 * (h @ W2)              W2 [4096, 1024]

Strategy: pure data-parallel over tokens across 8 NeuronCores (1024
tokens/core). Per core, activations live in SBUF as [features, tokens]
(features on partitions) so layers chain with no transposes.

Precision plan (PE roofline is the bottleneck; measured 221 ns per
128-contraction fp16/bf16 matmul vs 225 ns per 256-contraction fp8e4
DoubleRow matmul => DR is 1.97x): run the first N_DR*256 contraction rows
of layer 2 in fp8e4 DoubleRow (moving pairs = 2 k-planes per partition),
everything else in fp16 (exact for +-1 weights, ~2x less quantization
noise than bf16 on x/h). e4m3 quantization of h1 costs ~2.67e-2 rel err
per fully-quantized layer; with N_DR=8 of 16 tiles the simulated
end-to-end rel err is 1.89e-2 against the 2e-2 gate, and the DR half of
L2 runs at ~2x. DoubleRow with fp8 is exact for +-1 weights (products
pass through, fp32 accumulate), so the only error source is the e4m3
eviction rounding, which matches ml_dtypes RNE bit-for-bit (validated on
HW).
"""

from contextlib import ExitStack

import ml_dtypes
import numpy as np

P = 128
TOKENS = 8192
D_IN = 1024
D_H = 4096
D_OUT = 1024
N_CORES = 8
TOK_PER_CORE = TOKENS // N_CORES  # 1024
TOK_TILE = 512
NT = TOK_PER_CORE // TOK_TILE  # 2

N_DR = 8  # 256-row fp8 DoubleRow tiles in L2 (of 16); rest fp16
EXTRA_DR_T0 = True  # 9th DR tile, applied to the t=0 token half only
N_DR_W = N_DR + (1 if EXTRA_DR_T0 else 0)  # DR weight tiles to pack/stream
N_H1Q = 2 * N_DR  # L1 output strips evicted to fp8 pair-tiles (both halves)
N_H1H = D_H // P - N_H1Q  # L1 output strips evicted to fp16
N_WARM = 14  # dummy warm-up matmuls to lift the PE HAM clock-gate early

F16 = np.float16
F8 = ml_dtypes.float8_e4m3  # IEEE-style, max 240 == TRN float8e4

# Set TRACE=True (from test.py) to profile; LAST_EXEC_TIME_NS then holds the
# max per-core HW exec time of the most recent kernel() call.
TRACE = False
TRACE_CORES = None
LAST_EXEC_TIME_NS = None
LAST_RESULT = None

_cache = {}


def _raw_matmul(nc, out, lhsT, rhs, perf_mode=None, start=True, stop=True):
    """nc.tensor.matmul minus the fp8e4/e5-only perf-mode dtype assert."""
    import concourse.mybir as mybir

    eng = nc.tensor
    keep_dims = {0}
    if perf_mode is not None:
        keep_dims.add(1)
    ifmap_ap = eng.lower_ap(rhs.opt(keep_dims), opt=False)
    weights_ap = eng.lower_ap(lhsT.opt(keep_dims), opt=False,
                              for_matmul_weights=True)
    out_ap = eng.lower_ap(out)
    return eng.add_instruction(
        mybir.InstMatmult(
            name=nc.get_next_instruction_name(),
            replication_resolution=0,
            replication_shift_amnt=0,
            replication_num_rows=0,
            start_tensor_calc=start,
            stop_tensor_calc=stop,
            ins=[ifmap_ap, weights_ap],
            outs=[out_ap],
            perf_mode=perf_mode,
            is_transpose=None,
            ifmap_quant_offset=None,
            weights_quant_offset=None,
            bass_skip_group_check=False,
            tile_position=(0, 0),
            tile_size=(128, 128),
        )
    )


def _prune_dma_waits(nc, max_waits=1):
    """Drop transitively-implied waits from DMA instructions.

    DMA queue-entry descriptors hold a single sync wait; Tile's sem
    assignment is per-proc minimal but not transitively minimal across
    procs, so a recycled SBUF slot's DMA can carry WAR (engine) + WAW
    (prev slot writer's DMA lane) + lane-recycle waits = 3. The WAW (and
    often the recycle) wait is implied by the engine wait: the readers
    counted by the WAR threshold themselves waited on those DMAs.

    Soundness: a wait (s >= v) on instruction I is dropped only when the
    completion clocks implied by I's *other* waits already guarantee
    cumulative increments of s reached v. Completion clocks are built
    forward over the scheduled BIR order giving same-stream predecessor
    credit only to in-order engines (PE/ACT/DVE/SP), never to DMA lanes
    or Pool. Unrecognized wait/update modes contribute no credit, so
    unknowns can only inhibit pruning, never enable it.
    """
    import bisect

    import bass_rust

    IN_ORDER_ENGINES = {
        "EngineType.PE",
        "EngineType.Activation",
        "EngineType.DVE",
        "EngineType.SP",
    }

    sem_hist = {}  # sem -> ([cumulative values], [clocks at completion])
    sem_cum = {}  # sem -> cumulative increments so far
    eng_clock = {}  # engine -> completion clock of last instruction
    poisoned = set()  # sems with non-monotonic updates: no credit

    def cc(sem, val):
        """Completion clock implied by observing sem >= val, or None."""
        if sem in poisoned:
            return None
        hist = sem_hist.get(sem)
        if not hist or hist[0][-1] < val:
            return None
        return hist[1][bisect.bisect_left(hist[0], val)]

    def merge(dst, src):
        for k, v in src.items():
            if dst.get(k, 0) < v:
                dst[k] = v

    pruned = 0
    for bb in nc.m.functions[0].blocks:
        for inst in bb.instructions:
            si = inst.sync_info
            waits = list(si.on_wait or []) if si is not None else []
            ups = list(si.on_update or []) if si is not None else []
            is_dma = type(inst).__name__ == "InstDMACopy"

            clock = {}
            if not is_dma:
                prev = eng_clock.get(str(inst.engine))
                if prev is not None and str(inst.engine) in IN_ORDER_ENGINES:
                    merge(clock, prev)
            for w in waits:
                if w.wait_mode == "sem-ge-imm" and w.wait_value is not None:
                    c = cc(w.ant_name, w.wait_value)
                    if c is not None:
                        merge(clock, c)

            # Per-encoding wait budgets: DMA queue entries hold 1 wait;
            # engine instructions hold 2. Drain/EventSemaphore/control flow
            # are lowered specially by walrus — leave them alone.
            tname = type(inst).__name__
            if is_dma:
                cap = max_waits
            elif tname in ("InstDrain", "InstEventSemaphore", "InstCall",
                           "InstUnconditionalBranch", "InstISA"):
                cap = None
            else:
                cap = 2

            if cap is not None and len(waits) > cap:
                kept = list(waits)
                changed = True
                while len(kept) > cap and changed:
                    changed = False
                    for w in list(kept):
                        if w.wait_mode != "sem-ge-imm" or w.wait_value is None:
                            continue
                        implied = {}
                        provable = True
                        for o in kept:
                            if o is w:
                                continue
                            if o.wait_mode != "sem-ge-imm" or o.wait_value is None:
                                provable = False
                                break
                            c = cc(o.ant_name, o.wait_value)
                            if c is None:
                                provable = False
                                break
                            merge(implied, c)
                        if provable and implied.get(w.ant_name, 0) >= w.wait_value:
                            kept.remove(w)
                            pruned += 1
                            changed = True
                            break
                # Anything still over budget is left for Bacc's
                # generate_event_semaphores pass to split legally.
                if len(kept) != len(waits):
                    inst.sync_info = bass_rust.SyncInfo(on_wait=kept, on_update=ups)

            own = {}
            for u in ups:
                if u.update_mode not in ("sem-inc", "sem-add-imm"):
                    poisoned.add(u.ant_name)
                    continue
                inc = 1 if u.update_mode == "sem-inc" else u.update_value
                if inc is None:
                    poisoned.add(u.ant_name)
                    continue
                sem = u.ant_name
                sem_cum[sem] = sem_cum.get(sem, 0) + inc
                own[sem] = sem_cum[sem]
            merge(clock, own)
            for sem, cum in own.items():
                vals, clocks = sem_hist.setdefault(sem, ([], []))
                vals.append(cum)
                clocks.append(clock)
            if not is_dma:
                eng_clock[str(inst.engine)] = clock
    return pruned


def _build(a0, a1, a2):
    """Build the SPMD single-core program (same NEFF on all 8 cores)."""
    import concourse.mybir as mybir
    import concourse.tile as tile
    from concourse import bacc

    # Bacc (not plain Bass): its finalize() runs the wait-legalization
    # passes (move_matmul_waits_to_ldweights, generate_event_semaphores)
    # that split multi-wait instructions to the 1-wait HW encoding.
    nc = bacc.Bacc(
        "TRN2",
        target_bir_lowering=False,
        debug=False,
        enable_asserts=False,
        num_devices=N_CORES,
    )
    f16 = mybir.dt.float16
    f8 = mybir.dt.float8e4
    f32 = mybir.dt.float32
    DRM = mybir.MatmulPerfMode.DoubleRow

    xt = nc.dram_tensor("xt", [D_IN, TOK_PER_CORE], f16, kind="ExternalInput")
    w0p = nc.dram_tensor("w0p", [D_H // P, P, D_IN], f16, kind="ExternalInput")
    w1q = nc.dram_tensor("w1q", [D_H // P, P, N_DR_W * 256], f8,
                         kind="ExternalInput")
    w1h = nc.dram_tensor("w1h", [D_H // P, P, N_H1H * P], f16,
                         kind="ExternalInput")
    w2p = nc.dram_tensor("w2p", [D_OUT // P, P, D_H], f16, kind="ExternalInput")
    outt = nc.dram_tensor("outt", [D_OUT, TOK_PER_CORE], f32, kind="ExternalOutput")

    relu = mybir.ActivationFunctionType.Relu

    def tsl(t):
        return slice(t * TOK_TILE, (t + 1) * TOK_TILE)

    with tile.TileContext(nc) as tc, ExitStack() as ctx:
        xpool = ctx.enter_context(tc.tile_pool(name="xp", bufs=1))
        h1pool = ctx.enter_context(tc.tile_pool(name="h1p", bufs=1))
        h2pool = ctx.enter_context(tc.tile_pool(name="h2p", bufs=1))
        wpool = ctx.enter_context(tc.tile_pool(name="wp", bufs=4))
        w0pool = ctx.enter_context(tc.tile_pool(name="w0pl", bufs=10))
        w2pool = ctx.enter_context(tc.tile_pool(name="w2p", bufs=2))
        opool = ctx.enter_context(tc.tile_pool(name="op", bufs=3))
        pspool = ctx.enter_context(tc.tile_pool(name="psp", bufs=8, space="PSUM"))

        # x as per-j half-tiles in consumption order (t=0 first). The t=0
        # half is split across the ACT queue (j 0-3) and the SP queue
        # (j 4-7, behind only w0 strip 0) so it lands in ~half the time;
        # the t=1 half follows on ACT. (gpsimd's queue is unusable here:
        # ~10us cold init, and its teardown drain lengthens the tail.)
        x_half = [[None] * NT for _ in range(D_IN // P)]
        with tc.high_priority():
            # Pin the startup-critical transfers to the front of their
            # queues: without this the scheduler interleaves them behind
            # later DMAs and the first chain stalls until ~11us.
            w0_first = w0pool.tile([P, D_IN], f16, tag="w0", name="w0_0_0")
            nc.sync.dma_start(out=w0_first[:], in_=w0p[0])
            for t in range(NT):
                for j in range(D_IN // P):
                    h = xpool.tile([P, TOK_TILE], f16, tag=f"x{j}_{t}",
                                   name=f"x_{j}_{t}")
                    q = nc.sync if (t == 0 and j >= 4) else nc.scalar
                    q.dma_start(out=h[:], in_=xt[j * P : (j + 1) * P, tsl(t)])
                    x_half[j][t] = h

        # Warm-up: dummy matmuls on the first x tile while the rest of the
        # startup DMA is in flight. The PE HAM clock-gate only lifts to
        # 8/8 after ~3.4us of sustained busy; without these the first
        # ~20us of real matmuls run at 1.2 GHz (measured ~10us penalty).
        ps_warm = pspool.tile([P, TOK_TILE], f32, tag="ps", name="ps_warm")
        for i in range(N_WARM):
            nc.tensor.matmul(
                ps_warm[:], x_half[0][0][:, 0:P], x_half[0][0][:],
                start=True, stop=True,
            )

        # h1: first N_DR pair-tiles hold fp8 planes (L1 strips 2r, 2r+1),
        # rest fp16. h2: all fp16. With EXTRA_DR_T0, strips 16/17 are
        # additionally evicted to a t=0-only fp8 pair-tile (h1q8).
        h1q_tiles = [
            h1pool.tile([P, 2, TOK_PER_CORE], f8, tag=f"h1q_{r}", name=f"h1q_{r}")
            for r in range(N_DR)
        ]
        h1q8 = (
            h1pool.tile([P, 2, TOK_TILE], f8, tag="h1q8", name="h1q8")
            if EXTRA_DR_T0 else None
        )
        h1h_tiles = [
            h1pool.tile([P, TOK_PER_CORE], f16, tag=f"h1h_{j}", name=f"h1h_{j}")
            for j in range(N_H1H)
        ]
        h2_tiles = [
            h2pool.tile([P, TOK_PER_CORE], f16, tag=f"h2_{n}", name=f"h2_{n}")
            for n in range(D_H // P)
        ]

        # ---- Layer 1 (fp16). The first T0_FIRST strips run their t=0
        # chain only (the t=0 half of x is all they need, so sustained
        # compute starts while the t=1 half is still in flight), then
        # their t=1 chains, then the rest t_outer per strip. Only the
        # first T0_FIRST w0 strips are streamed twice (+2 MB).
        T0_FIRST = 8

        def l1_evict(n, t):
            if n < N_H1Q:
                return h1q_tiles[n // 2][:, n % 2, tsl(t)]
            if EXTRA_DR_T0 and t == 0 and n < N_H1Q + 2:
                return h1q8[:, n - N_H1Q, :]
            return h1h_tiles[n - N_H1Q][:, tsl(t)]

        def l1_chain(w, n, t, jorder=None):
            jorder = jorder or list(range(D_IN // P))
            ps = pspool.tile([P, TOK_TILE], f32, tag="ps", name=f"ps0_{n}_{t}")
            for i, j in enumerate(jorder):
                nc.tensor.matmul(
                    ps[:],
                    w[:, j * P : (j + 1) * P],
                    x_half[j][t][:],
                    start=(i == 0),
                    stop=(i == D_IN // P - 1),
                )
            nc.scalar.activation(l1_evict(n, t), ps[:], relu, scale=a0)

        def l1_w(n, t):
            if n == 0 and t == 0:
                return w0_first
            w = w0pool.tile([P, D_IN], f16, tag="w0", name=f"w0_{n}_{t}")
            nc.sync.dma_start(out=w[:], in_=w0p[n])
            return w

        # The first chains consume x tiles in their DMA-landing order
        # (j 0-3 on ACT, j 4-7 on SP, interleaved by arrival) so the first
        # chain is never j-blocked while later tiles stream in.
        J_ARRIVAL = [0, 4, 1, 5, 2, 6, 3, 7]
        for n in range(T0_FIRST):
            l1_chain(l1_w(n, 0), n, 0, jorder=J_ARRIVAL)
        for n in range(T0_FIRST, D_H // P):
            w = l1_w(n, 0)
            for t in range(NT):
                l1_chain(w, n, t)
        # t=1 chains of the first strips run LAST: their w0 re-streams ride
        # the SP queue during L1's long slack window instead of colliding
        # with the t=0 w0 stream (measured 2.1us PE gap + HAM re-throttle).
        for n in range(T0_FIRST):
            l1_chain(l1_w(n, 1), n, 1)

        # ---- Layer 2: N_DR fp8-DoubleRow 256-row tiles + N_H1H fp16
        # 128-row tiles per accumulation chain. t-inner alternates PSUM
        # banks, which measures ~0.7 ns/MM faster than same-bank runs.
        for n in range(D_H // P):
            wq = wpool.tile([P, N_DR_W * 256], f8, tag="w1q", name=f"w1q_{n}")
            # ACT queue: keeps the L2 fp8 stream off the SP queue, which
            # otherwise delays the L1 w0 strips (measured 5us LDW stall).
            nc.scalar.dma_start(out=wq[:], in_=w1q[n])
            wh = wpool.tile([P, N_H1H * P], f16, tag="w1h", name=f"w1h_{n}")
            # ACT queue too: early w1h strips on SP delayed the L1 w0
            # stream (~2us LDW stalls at the t_outer transition).
            nc.scalar.dma_start(out=wh[:], in_=w1h[n])
            pss = [
                pspool.tile([P, TOK_TILE], f32, tag="ps", name=f"ps1_{n}_{t}")
                for t in range(NT)
            ]
            for r in range(N_DR_W):
                wap = wq[:, r * 256 : (r + 1) * 256].rearrange(
                    "p (two c) -> p two c", two=2
                )
                for t in range(NT):
                    if r == N_DR:  # t=0-only extra tile
                        if t != 0:
                            continue
                        mv = h1q8[:, :, :]
                    else:
                        mv = h1q_tiles[r][:, :, tsl(t)]
                    _raw_matmul(
                        nc, pss[t][:], wap, mv,
                        perf_mode=DRM, start=(r == 0), stop=False,
                    )
            for j in range(N_H1H):
                for t in range(NT):
                    if EXTRA_DR_T0 and t == 0 and j < 2:
                        continue  # rows 2048..2303 covered by the DR tile
                    nc.tensor.matmul(
                        pss[t][:],
                        wh[:, j * P : (j + 1) * P],
                        h1h_tiles[j][:, tsl(t)],
                        start=(N_DR == 0 and j == 0),
                        stop=(j == N_H1H - 1),
                    )
            for t in range(NT):
                nc.scalar.activation(h2_tiles[n][:, tsl(t)], pss[t][:], relu,
                                     scale=a1)

        # ---- Layer 3 (fp16), streaming the output out per token half.
        for n in range(D_OUT // P):
            w = w2pool.tile([P, D_H], f16, tag="w2", name=f"w2_{n}")
            nc.sync.dma_start(out=w[:], in_=w2p[n])
            pss = [
                pspool.tile([P, TOK_TILE], f32, tag="ps", name=f"ps2_{n}_{t}")
                for t in range(NT)
            ]
            def l3_evict(t):
                # Chunked evictions on alternating engines (ACT / DVE) with
                # the output DMA split over two queues: the final strip's
                # eviction+DMA pipeline shortens the kernel tail.
                half = TOK_TILE // 2
                for c in range(2):
                    o = opool.tile([P, half], f32, tag="o", name=f"o_{n}_{t}_{c}")
                    src = pss[t][:, c * half : (c + 1) * half]
                    if (2 * t + c) % 2 == 0:
                        nc.scalar.mul(o[:], src, a2)
                    else:
                        nc.vector.tensor_scalar_mul(o[:], src, a2)
                    dq = nc.scalar if c % 2 == 0 else nc.sync
                    dq.dma_start(
                        out=outt[n * P : (n + 1) * P,
                                 t * TOK_TILE + c * half : t * TOK_TILE + (c + 1) * half],
                        in_=o[:],
                    )

            # Last strip runs t_outer: its t=0 eviction+DMA overlap the
            # t=1 chain, so only one half's eviction trails the final MM.
            last = n == D_OUT // P - 1
            if last:
                order = [(t, j) for t in range(NT) for j in range(D_H // P)]
            else:
                order = [(t, j) for j in range(D_H // P) for t in range(NT)]
            for t, j in order:
                nc.tensor.matmul(
                    pss[t][:],
                    w[:, j * P : (j + 1) * P],
                    h2_tiles[j][:, tsl(t)],
                    start=(j == 0),
                    stop=(j == D_H // P - 1),
                )
                if last and j == D_H // P - 1:
                    l3_evict(t)
            if not last:
                for t in range(NT):
                    l3_evict(t)

    _prune_dma_waits(nc)
    nc.finalize()
    return nc


def _pack_w_f16(k):
    """Bool [K, N] -> f16 +-1 packed [N/P, P, K]: strip n, partition p,
    free j*P+c  <-  W[j*P+p, n*P+c] (partition = contraction for lhsT)."""
    K, N = k.shape
    w = np.where(k, np.float32(1.0), np.float32(-1.0)).astype(F16)
    return np.ascontiguousarray(
        w.reshape(K // P, P, N // P, P).transpose(2, 1, 0, 3).reshape(N // P, P, K)
    )


def _pack_w1(k1):
    """W1 split: rows [0, N_DR_W*256) -> fp8 DoubleRow pair layout
    [N/P, P, r*256 + plane*128 + c] <- W[256r + 128*plane + p, n*128 + c];
    rows [N_DR*256, 4096) -> fp16 strip layout like _pack_w_f16. With
    EXTRA_DR_T0 rows 2048..2303 are packed in BOTH (fp8 serves the t=0
    token half, fp16 the t=1 half)."""
    w = np.where(k1, np.float32(1.0), np.float32(-1.0))
    wa = w[: N_DR_W * 256].astype(F8)
    wb = w[N_DR * 256 :].astype(F16)
    n_t = wa.shape[1] // P
    # [r, plane, p, n, c] -> [n, p, r, plane, c]
    w1q = np.ascontiguousarray(
        wa.reshape(N_DR_W, 2, P, n_t, P).transpose(3, 2, 0, 1, 4).reshape(n_t, P, -1)
    )
    kb = wb.shape[0]
    w1h = np.ascontiguousarray(
        wb.reshape(kb // P, P, n_t, P).transpose(2, 1, 0, 3).reshape(n_t, P, kb)
    )
    return w1q, w1h


def _enable_ntff_trace():
    """Best-effort plumbing for trace=True under axon in this image.

    The image's ``antenv`` lacks the ``axon_hooks`` shim that
    ``trn_agent_boot`` would normally register the NTFF profile hook
    into, and there is no artifact bucket — stub both.
    """
    import sys
    import types

    import concourse.bass_utils as bu

    bu.upload_artifacts = lambda tmpdir: tmpdir
    try:
        from antenv import axon_hooks
    except ImportError:
        import antenv

        axon_hooks = types.ModuleType("antenv.axon_hooks")
        _state = {"hook": None}
        axon_hooks.set_axon_ntff_profile_hook = lambda h: _state.__setitem__(
            "hook", h
        )
        axon_hooks.get_axon_ntff_profile_hook = lambda: _state["hook"]
        sys.modules["antenv.axon_hooks"] = axon_hooks
        antenv.axon_hooks = axon_hooks
    if axon_hooks.get_axon_ntff_profile_hook() is None:
        from trn_agent_boot.trn_boot import _ntff_profile_via_ctypes

        axon_hooks.set_axon_ntff_profile_hook(
            _ntff_profile_via_ctypes("/opt/axon/libaxon_pjrt.so")
        )


def kernel(x, k0, k1, k2, s0, s1, s2):
    global LAST_EXEC_TIME_NS, LAST_RESULT
    from concourse.bass_utils import run_bass_kernel_spmd

    if TRACE:
        _enable_ntff_trace()

    x = np.asarray(x)
    a0 = 2.0 * float(np.asarray(s0))
    a1 = 2.0 * float(np.asarray(s1))
    a2 = float(np.asarray(s2))

    key = (a0, a1, a2)
    if key not in _cache:
        _cache[key] = _build(a0, a1, a2)
    nc = _cache[key]

    w0p = _pack_w_f16(np.asarray(k0))
    w1qp, w1hp = _pack_w1(np.asarray(k1))
    w2p = _pack_w_f16(np.asarray(k2))

    in_maps = []
    for i in range(N_CORES):
        xs = x[i * TOK_PER_CORE : (i + 1) * TOK_PER_CORE].astype(F16)
        in_maps.append(
            {
                "xt": np.ascontiguousarray(xs.T),
                "w0p": w0p,
                "w1q": w1qp,
                "w1h": w1hp,
                "w2p": w2p,
            }
        )

    res = run_bass_kernel_spmd(
        nc, in_maps, list(range(N_CORES)), trace=TRACE, trace_cores=TRACE_CORES
    )
    LAST_EXEC_TIME_NS = res.exec_time_ns
    LAST_RESULT = res
    out = np.concatenate(
        [res.results[i]["outt"].T for i in range(N_CORES)], axis=0
    )
    return np.ascontiguousarray(out)
